# revision 1
# baseline (speedup 1.0000x reference)
"""Trainium2 Bass kernel for nn_Attention_v2_cross (dense transformer, 8 cores).

Sharding: 8 cores = 4 batches x 2 query-halves (data parallel over batch and
query positions). Every core holds the full weights and the full context for
its batch, so attention needs no cross-device communication; the kv projection
is duplicated across the two cores sharing a batch (+25% matmul flops, zero
collectives).

Per-core pipeline (all matmuls in fp32r = single-pass FP22, full PE rate at
free-dim >= 256):
  1. LN(x) folded into the q projection: stats per row via ones-matmul on the
     transposed activations, apply as x*a+b with a,b broadcast across
     partitions via K=1 matmuls.
  2. q/k projected output-transposed ([e, row]); v projected row-major
     ([row, e]) - that orientation split is what the sim and attn@v matmuls
     need, and both projections cost the same.
  3. Per head: sim = qT.T @ kT (K=64 per r, accumulated over r=12), row-max,
     exp((sim-max)*128), row-sum, normalize, PE-transpose the normalized P
     tiles, then attn@v with v as the stationary operand (M=64).
  4. Out projection from the attention output (staged transposed in DRAM),
     then the final layernorm row-major.
"""

import os
import numpy as np

B, N, R, C = 4, 1024, 12, 512
H, D = 8, 64
E = H * D            # 512
NQ = N // 2          # 512 queries per core
NKJ = N              # 1024 keys per core
ALPHA = 128.0
EPS = 1e-5
XCOLS = R * NQ       # 6144  (col = r*NQ + i)
CCOLS = R * NKJ      # 12288 (col = r*NKJ + j)
P = 128

_CACHE = {}


def _build_program():
    from contextlib import ExitStack
    import concourse.bass as bass
    import concourse.tile as tile
    from concourse import bacc
    from concourse import mybir
    from concourse.masks import make_identity

    F32 = mybir.dt.float32
    F32R = mybir.dt.float32r
    AF = mybir.ActivationFunctionType
    AX = mybir.AxisListType.X

    nc = bacc.Bacc("TRN2", target_bir_lowering=False, debug=False, num_devices=8)

    xT = nc.dram_tensor("xT", [C, XCOLS], F32R, kind="ExternalInput").ap()
    ctxT = nc.dram_tensor("ctxT", [C, CCOLS], F32R, kind="ExternalInput").ap()
    wqT = nc.dram_tensor("wqT", [C, E], F32R, kind="ExternalInput").ap()
    wkT = nc.dram_tensor("wkT", [C, E], F32R, kind="ExternalInput").ap()
    wvT = nc.dram_tensor("wvT", [C, E], F32R, kind="ExternalInput").ap()
    woT = nc.dram_tensor("woT", [E, C], F32R, kind="ExternalInput").ap()
    nullk = nc.dram_tensor("nullk", [D, 2], F32R, kind="ExternalInput").ap()
    onesc = nc.dram_tensor("onesc", [P, 1], F32R, kind="ExternalInput").ap()
    nullv = nc.dram_tensor("nullv", [1, D], F32R, kind="ExternalInput").ap()
    outg = nc.dram_tensor("outg", [1, C], F32, kind="ExternalInput").ap()
    out = nc.dram_tensor("out", [XCOLS, C], F32, kind="ExternalOutput").ap()

    with ExitStack() as ctx:
        tc = ctx.enter_context(tile.TileContext(nc))

        const = ctx.enter_context(tc.tile_pool(name="const", bufs=1))
        dram = ctx.enter_context(tc.tile_pool(name="dram", bufs=1, space="DRAM"))

        identity = const.tile([P, P], F32)
        make_identity(nc, identity[:])
        ones_col = const.tile([P, 1], F32R)
        nc.sync.dma_start(ones_col[:], onesc[:, :])
        ones_row = const.tile([1, P], F32)
        nc.vector.memset(ones_row[:], 1.0)
        nullk_s = const.tile([P, 2], F32R)
        nc.sync.dma_start(nullk_s[0:D, :], nullk[:, :])
        nc.sync.dma_start(nullk_s[D : 2 * D, :], nullk[:, :])
        nullv_s = const.tile([1, D], F32R)
        nc.sync.dma_start(nullv_s[:, :], nullv[:, :])
        outg_s = const.tile([P, C], F32)
        nc.sync.dma_start(outg_s[:, :], outg.to_broadcast((P, C)))
        eps_1 = const.tile([1, 1], F32)
        nc.vector.memset(eps_1[:], EPS)
        eps_P = const.tile([P, 1], F32)
        nc.vector.memset(eps_P[:], EPS)

        qT_d = dram.tile([P, 4, XCOLS], F32R)    # qT[e, col]: e = ec*128+p
        kT_d = dram.tile([P, 4, CCOLS], F32R)
        vM_d = dram.tile([P, CCOLS // P, E], F32R)  # v rows (r,j): row = rc*128+p
        aoT_d = dram.tile([P, 4, XCOLS], F32R)

        # ---------------- Stage 1: projections -------------------------
        with tc.tile_pool(name="w1", bufs=1) as wpool, \
             tc.tile_pool(name="s1", bufs=8) as s1, \
             tc.tile_pool(name="s1b", bufs=4) as s1b, \
             tc.tile_pool(name="p1", bufs=2, space="PSUM") as p1, \
             tc.tile_pool(name="p1s", bufs=1, space="PSUM") as p1s:

            wq_s = wpool.tile([P, 4, E], F32R)
            wk_s = wpool.tile([P, 4, E], F32R)
            wv_s = wpool.tile([P, 4, E], F32R)
            for cc in range(4):
                nc.sync.dma_start(wq_s[:, cc, :], wqT[cc * P : (cc + 1) * P, :])
                nc.sync.dma_start(wk_s[:, cc, :], wkT[cc * P : (cc + 1) * P, :])
                nc.sync.dma_start(wv_s[:, cc, :], wvT[cc * P : (cc + 1) * P, :])

            # ---- 1a: LN(x) + q projection (transposed out) ----
            for rb in range(R):
                xts = []
                for cc in range(4):
                    t = s1.tile([P, NQ], F32R, tag="xt")
                    nc.sync.dma_start(t[:], xT[cc * P : (cc + 1) * P, rb * NQ : (rb + 1) * NQ])
                    xts.append(t)
                psum_sum = p1s.tile([1, NQ], F32, tag="stat_sum")
                psum_sq = p1s.tile([1, NQ], F32, tag="stat_sq")
                for cc in range(4):
                    nc.tensor.matmul(psum_sum[:], ones_col[:].bitcast(F32R),
                                     xts[cc][:].bitcast(F32R),
                                     start=(cc == 0), stop=(cc == 3))
                sqs = []
                for cc in range(4):
                    sq = s1.tile([P, NQ], F32R, tag="sq")
                    nc.scalar.activation(sq[:], xts[cc][:].bitcast(F32), AF.Square)
                    sqs.append(sq)
                for cc in range(4):
                    nc.tensor.matmul(psum_sq[:], ones_col[:].bitcast(F32R),
                                     sqs[cc][:].bitcast(F32R),
                                     start=(cc == 0), stop=(cc == 3))
                mean = s1b.tile([1, NQ], F32, tag="mean")
                nc.scalar.mul(mean[:], psum_sum[:], 1.0 / C)
                msq = s1b.tile([1, NQ], F32, tag="msq")
                nc.scalar.activation(msq[:], mean[:], AF.Square)
                var = s1b.tile([1, NQ], F32, tag="var")
                nc.scalar.mul(var[:], psum_sq[:], 1.0 / C)
                nc.vector.tensor_sub(var[:], var[:], msq[:])
                std = s1b.tile([1, NQ], F32, tag="std")
                nc.scalar.activation(std[:], var[:], AF.Sqrt, bias=eps_1[:])
                inv = s1b.tile([1, NQ], F32, tag="inv")
                nc.vector.reciprocal(inv[:], std[:])
                negb = s1b.tile([1, NQ], F32, tag="negb")
                nc.vector.tensor_mul(negb[:], mean[:], inv[:])
                nc.scalar.mul(negb[:], negb[:], -1.0)
                # broadcast a (=inv) and b (=-mean*inv) across partitions via K=1 matmul
                a_b = p1s.tile([P, NQ], F32, tag="a_b")
                nc.tensor.matmul(a_b[:], ones_row[:], inv[:], start=True, stop=True)
                b_b = p1s.tile([P, NQ], F32, tag="b_b")
                nc.tensor.matmul(b_b[:], ones_row[:], negb[:], start=True, stop=True)
                xns = []
                for cc in range(4):
                    xn = s1.tile([P, NQ], F32R, tag="xn")
                    nc.vector.tensor_mul(xn[:], xts[cc][:].bitcast(F32), a_b[:])
                    nc.vector.tensor_add(xn[:], xn[:].bitcast(F32), b_b[:])
                    xns.append(xn)
                for ec in range(4):
                    pq = p1.tile([P, NQ], F32, tag="proj")
                    for cc in range(4):
                        nc.tensor.matmul(
                            pq[:],
                            wq_s[:, cc, ec * P : (ec + 1) * P].bitcast(F32R),
                            xns[cc][:].bitcast(F32R),
                            start=(cc == 0), stop=(cc == 3))
                    qs = s1b.tile([P, NQ], F32R, tag="qstage")
                    nc.any.tensor_copy(qs[:], pq[:])
                    nc.sync.dma_start(qT_d[:, ec, rb * NQ : (rb + 1) * NQ], qs[:])

            # ---- 1b: k projection (transposed) + v projection (row-major) ----
            for cb in range(CCOLS // NQ):  # 24 blocks of 512 context columns
                cts = []
                for cc in range(4):
                    t = s1.tile([P, NQ], F32R, tag="ct")
                    nc.sync.dma_start(t[:], ctxT[cc * P : (cc + 1) * P, cb * NQ : (cb + 1) * NQ])
                    cts.append(t)
                for ec in range(4):
                    pk = p1.tile([P, NQ], F32, tag="proj")
                    for cc in range(4):
                        nc.tensor.matmul(
                            pk[:],
                            wk_s[:, cc, ec * P : (ec + 1) * P].bitcast(F32R),
                            cts[cc][:].bitcast(F32R),
                            start=(cc == 0), stop=(cc == 3))
                    ks = s1b.tile([P, NQ], F32R, tag="kstage")
                    nc.any.tensor_copy(ks[:], pk[:])
                    nc.sync.dma_start(kT_d[:, ec, cb * NQ : (cb + 1) * NQ], ks[:])
                for rc4 in range(4):
                    pv = p1.tile([P, E], F32, tag="proj")
                    for cc in range(4):
                        nc.tensor.matmul(
                            pv[:],
                            cts[cc][:, rc4 * P : (rc4 + 1) * P].bitcast(F32R),
                            wv_s[:, cc, :].bitcast(F32R),
                            start=(cc == 0), stop=(cc == 3))
                    vs = s1b.tile([P, E], F32R, tag="vstage")
                    nc.any.tensor_copy(vs[:], pv[:])
                    nc.sync.dma_start(vM_d[:, cb * 4 + rc4, :], vs[:])

        # ---------------- Stage 2: attention ---------------------------
        with tc.tile_pool(name="kq2", bufs=1) as kq2, \
             tc.tile_pool(name="pt2", bufs=2) as pt2, \
             tc.tile_pool(name="s2", bufs=4) as s2, \
             tc.tile_pool(name="st2", bufs=6) as st2, \
             tc.tile_pool(name="v2", bufs=6) as v2, \
             tc.tile_pool(name="pa2", bufs=1, space="PSUM") as pa2, \
             tc.tile_pool(name="pb2", bufs=1, space="PSUM") as pb2, \
             tc.tile_pool(name="pc2", bufs=2, space="PSUM") as pc2:

            JC = NKJ // P  # 8 key chunks of 128
            for g in range(4):  # head pairs
                kpair = kq2.tile([P, CCOLS], F32R, tag="kpair")
                nc.sync.dma_start(kpair[:], kT_d[:, g, :])
                qpair = kq2.tile([P, XCOLS], F32R, tag="qpair")
                nc.sync.dma_start(qpair[:], qT_d[:, g, :])
                for hh in range(2):
                    h = 2 * g + hh
                    pb = hh * D  # partition base: 0 or 64
                    PT = pt2.tile([P, JC, NQ], F32R, tag="PT")
                    PnT = pt2.tile([1, NQ], F32R, tag="PnT")
                    for ib in range(NQ // P):  # 4 query blocks
                        ps = []
                        for jb in range(2):
                            pj = pa2.tile([P, NQ], F32, tag=f"sim{jb}")
                            for r in range(R):
                                nc.tensor.matmul(
                                    pj[:],
                                    qpair[pb : pb + D, r * NQ + ib * P : r * NQ + (ib + 1) * P].bitcast(F32R),
                                    kpair[pb : pb + D, r * NKJ + jb * NQ : r * NKJ + (jb + 1) * NQ].bitcast(F32R),
                                    start=(r == 0), stop=(r == R - 1))
                            ps.append(pj)
                        pn = pb2.tile([P, 2], F32, tag="simnull")
                        for r in range(R):
                            nc.tensor.matmul(
                                pn[:],
                                qpair[pb : pb + D, r * NQ + ib * P : r * NQ + (ib + 1) * P].bitcast(F32R),
                                nullk_s[pb : pb + D, :].bitcast(F32R),
                                start=(r == 0), stop=(r == R - 1))
                        m = st2.tile([P, 1], F32, tag="m")
                        m1 = st2.tile([P, 1], F32, tag="m1")
                        nc.vector.reduce_max(m[:], ps[0][:], axis=AX)
                        nc.vector.reduce_max(m1[:], ps[1][:], axis=AX)
                        nc.vector.tensor_max(m[:], m[:], m1[:])
                        nc.vector.tensor_max(m[:], m[:], pn[:, 0:1])
                        negm = st2.tile([P, 1], F32, tag="negm")
                        nc.scalar.mul(negm[:], m[:], -ALPHA)
                        e0 = s2.tile([P, NQ], F32, tag="e0")
                        e1 = s2.tile([P, NQ], F32, tag="e1")
                        nc.scalar.activation(e0[:], ps[0][:], AF.Exp, bias=negm[:], scale=ALPHA)
                        nc.scalar.activation(e1[:], ps[1][:], AF.Exp, bias=negm[:], scale=ALPHA)
                        en = st2.tile([P, 1], F32, tag="en")
                        nc.scalar.activation(en[:], pn[:, 0:1], AF.Exp, bias=negm[:], scale=ALPHA)
                        s0 = st2.tile([P, 1], F32, tag="s0")
                        s1r = st2.tile([P, 1], F32, tag="s1r")
                        nc.vector.reduce_sum(s0[:], e0[:], axis=AX)
                        nc.vector.reduce_sum(s1r[:], e1[:], axis=AX)
                        den = st2.tile([P, 1], F32, tag="den")
                        nc.vector.tensor_add(den[:], s0[:], s1r[:])
                        nc.vector.tensor_add(den[:], den[:], en[:])
                        dinv = st2.tile([P, 1], F32, tag="dinv")
                        nc.vector.reciprocal(dinv[:], den[:])
                        nc.vector.tensor_mul(e0[:], e0[:], dinv[:].to_broadcast((P, NQ)))
                        nc.vector.tensor_mul(e1[:], e1[:], dinv[:].to_broadcast((P, NQ)))
                        pnorm = st2.tile([P, 1], F32, tag="pnorm")
                        nc.vector.tensor_mul(pnorm[:], en[:], dinv[:])
                        for jb in range(2):
                            src = e0 if jb == 0 else e1
                            for c4 in range(4):
                                tp = pc2.tile([P, P], F32, tag="tp")
                                nc.tensor.transpose(tp[:], src[:, c4 * P : (c4 + 1) * P], identity[:])
                                nc.any.tensor_copy(PT[:, jb * 4 + c4, ib * P : (ib + 1) * P], tp[:])
                        tpn = pb2.tile([1, P], F32, tag="tpn")
                        nc.tensor.transpose(tpn[:], pnorm[:, :], identity[:])
                        nc.any.tensor_copy(PnT[:, ib * P : (ib + 1) * P], tpn[:])
                    # attn @ v for head h
                    for r in range(R):
                        pav = pb2.tile([D, NQ], F32, tag="pav")
                        for jc in range(JC):
                            vt = v2.tile([P, D], F32R, tag="vt")
                            nc.sync.dma_start(vt[:], vM_d[:, r * JC + jc, h * D : (h + 1) * D])
                            nc.tensor.matmul(
                                pav[:], vt[:].bitcast(F32R), PT[:, jc, :].bitcast(F32R),
                                start=(jc == 0), stop=False)
                        nc.tensor.matmul(
                            pav[:], nullv_s[:, :].bitcast(F32R), PnT[:, :].bitcast(F32R),
                            start=False, stop=True)
                        avs = s2.tile([D, NQ], F32R, tag="avstage")
                        nc.any.tensor_copy(avs[:], pav[:])
                        nc.sync.dma_start(
                            aoT_d[pb : pb + D, g, r * NQ : (r + 1) * NQ], avs[:])

        # ---------------- Stage 3: out projection + final LN ------------
        with tc.tile_pool(name="w3", bufs=1) as w3, \
             tc.tile_pool(name="s3", bufs=8) as s3, \
             tc.tile_pool(name="s3b", bufs=4) as s3b, \
             tc.tile_pool(name="st3", bufs=6) as st3, \
             tc.tile_pool(name="p3", bufs=4, space="PSUM") as p3:

            wo_s = w3.tile([P, 4, C], F32R)
            for ec in range(4):
                nc.sync.dma_start(wo_s[:, ec, :], woT[ec * P : (ec + 1) * P, :])

            for rc in range(XCOLS // P):  # 48 row chunks
                pf = p3.tile([P, C], F32, tag="pf")
                for ec in range(4):
                    at = s3.tile([P, P], F32R, tag="at")
                    nc.sync.dma_start(at[:], aoT_d[:, ec, rc * P : (rc + 1) * P])
                    nc.tensor.matmul(
                        pf[:], at[:].bitcast(F32R), wo_s[:, ec, :].bitcast(F32R),
                        start=(ec == 0), stop=(ec == 3))
                nmean = st3.tile([P, 1], F32, tag="nmean")
                nc.vector.reduce_sum(nmean[:], pf[:], axis=AX)
                nc.scalar.mul(nmean[:], nmean[:], -1.0 / C)
                cen = s3b.tile([P, C], F32, tag="cen")
                nc.scalar.add(cen[:], pf[:], nmean[:])
                sq3 = s3b.tile([P, C], F32, tag="sq3")
                nc.scalar.activation(sq3[:], cen[:], AF.Square)
                var3 = st3.tile([P, 1], F32, tag="var3")
                nc.vector.reduce_sum(var3[:], sq3[:], axis=AX)
                nc.scalar.mul(var3[:], var3[:], 1.0 / C)
                std3 = st3.tile([P, 1], F32, tag="std3")
                nc.scalar.activation(std3[:], var3[:], AF.Sqrt, bias=eps_P[:])
                inv3 = st3.tile([P, 1], F32, tag="inv3")
                nc.vector.reciprocal(inv3[:], std3[:])
                on = s3b.tile([P, C], F32, tag="on")
                nc.vector.tensor_mul(on[:], cen[:], inv3[:].to_broadcast((P, C)))
                nc.vector.tensor_mul(on[:], on[:], outg_s[:, :])
                nc.sync.dma_start(out[rc * P : (rc + 1) * P, :], on[:])

    nc.compile()
    return nc


def kernel(x, context, norm_g, to_q_w, to_kv_w, null_kv, to_out_w, out_norm_g):
    from concourse.bass_utils import run_bass_kernel_spmd

    x = np.asarray(x, dtype=np.float32)
    context = np.asarray(context, dtype=np.float32)
    norm_g = np.asarray(norm_g, dtype=np.float32)
    to_q_w = np.asarray(to_q_w, dtype=np.float32)
    to_kv_w = np.asarray(to_kv_w, dtype=np.float32)
    null_kv = np.asarray(null_kv, dtype=np.float32)
    to_out_w = np.asarray(to_out_w, dtype=np.float32)
    out_norm_g = np.asarray(out_norm_g, dtype=np.float32)

    if "nc" not in _CACHE:
        _CACHE["nc"] = _build_program()
    nc = _CACHE["nc"]

    scale = (D ** -0.5) / ALPHA * (R ** -0.5)
    wq = np.ascontiguousarray((to_q_w * norm_g[None, :] * scale).T)
    wk = np.ascontiguousarray(to_kv_w[:E].T)
    wv = np.ascontiguousarray(to_kv_w[E:].T)
    wo = np.ascontiguousarray(to_out_w.T)
    nullk_a = np.ascontiguousarray(np.repeat(null_kv[0].reshape(D, 1), 2, axis=1))
    nullv_a = np.ascontiguousarray(null_kv[1].reshape(1, D))
    outg_a = np.ascontiguousarray(out_norm_g.reshape(1, C))
    ones_a = np.ones((P, 1), dtype=np.float32)

    in_maps = []
    for core in range(8):
        bi, half = core // 2, core % 2
        xs = x[bi, half * NQ : (half + 1) * NQ]          # [512, 12, 512]
        xT_a = np.ascontiguousarray(xs.transpose(2, 1, 0).reshape(C, XCOLS))
        cs = context[bi]                                  # [1024, 12, 512]
        ctxT_a = np.ascontiguousarray(cs.transpose(2, 1, 0).reshape(C, CCOLS))
        in_maps.append(dict(
            xT=xT_a, ctxT=ctxT_a, wqT=wq, wkT=wk, wvT=wv, woT=wo,
            nullk=nullk_a, nullv=nullv_a, outg=outg_a, onesc=ones_a))

    trace = bool(int(os.environ.get("KERNEL_TRACE", "0")))
    res = run_bass_kernel_spmd(nc, in_maps, list(range(8)), trace=trace)
    _CACHE["last_exec_ns"] = res.exec_time_ns

    outs = []
    for core in range(8):
        o = res.results[core]["out"]                      # [6144, 512], rows (r, i)
        outs.append(o.reshape(R, NQ, C).transpose(1, 0, 2))  # [512, 12, 512]
    full = np.stack(
        [np.concatenate([outs[2 * bi], outs[2 * bi + 1]], axis=0) for bi in range(B)])
    return full.astype(np.float32)



# revision 22
# speedup vs baseline: 4.8116x; 4.8116x over previous
"""Trainium2 Bass kernel for nn_Attention_v2_cross (dense transformer, 8 cores).

Sharding: 8 cores = 4 batches x 2 query-halves (data parallel over batch and
query positions). Every core holds the full weights and the full context for
its batch, so attention needs no cross-device communication; the kv projection
is duplicated across the two cores sharing a batch (zero collectives).

Host passes x/context slices in their NATIVE layout (views, no transposes);
all re-layout happens on device:
  1. x tiles [128 rows, C] are layernormed row-major (bn_stats on DVE, fused
     scale/bias apply on ACT, cast to bf16) and PE-transposed into [C, rows]
     for the projections. ctx tiles likewise (no LN).
  2. q/k/v projected in f32r/bf16; k stays SBUF-resident, q and v staged to
     DRAM in bf16 with large contiguous DMAs.
  3. Attention per head-pair: sim accumulated over r with K=64 matmuls
     interleaved across the two heads (PE row-tiling runs them concurrently
     on HW); softmax skips the max-subtraction (alpha*sim ~ N(0, 0.2), and
     softmax is shift invariant) with exp row-sums via ACT accum_out; P is
     transposed per 128-block in bf16; attn@v packs the two r-parities of v
     into the stationary operand (M=128) since attention weights do not
     depend on r; 1/denominator is folded into the PSUM->SBUF copy of the
     attention output.
  4. Out projection from bf16-staged aoT, final LN row-major, output written
     in natural (i, r) row order so the host does no transposes.
"""

import os
import numpy as np

B, N, R, C = 4, 1024, 12, 512
H, D = 8, 64
E = H * D            # 512
NQ = N // 2          # 512 queries per core
NKJ = N              # 1024 keys per core
ALPHA = 128.0
EPS = 1e-5
XCOLS = R * NQ       # 6144
CCOLS = R * NKJ      # 12288
P = 128

_CACHE = {}


def _build_program():
    from contextlib import ExitStack
    import concourse.bass as bass
    import concourse.tile as tile
    from concourse import bacc
    from concourse import mybir
    from concourse.masks import make_identity

    F32 = mybir.dt.float32
    F32R = mybir.dt.float32r
    BF16 = mybir.dt.bfloat16
    AF = mybir.ActivationFunctionType

    nc = bacc.Bacc("TRN2", target_bir_lowering=False, debug=False, num_devices=8)

    xR = nc.dram_tensor("xR", [NQ, R, C], F32, kind="ExternalInput").ap()
    ctxR = nc.dram_tensor("ctxR", [NKJ, R, C], F32, kind="ExternalInput").ap()
    wqT = nc.dram_tensor("wqT", [C, E], BF16, kind="ExternalInput").ap()
    wkT = nc.dram_tensor("wkT", [C, E], BF16, kind="ExternalInput").ap()
    wvT = nc.dram_tensor("wvT", [C, E], BF16, kind="ExternalInput").ap()
    woT = nc.dram_tensor("woT", [E, C], BF16, kind="ExternalInput").ap()
    nullkD = nc.dram_tensor("nullkD", [P, 2], BF16, kind="ExternalInput").ap()
    nullv2 = nc.dram_tensor("nullv2", [1, P], BF16, kind="ExternalInput").ap()
    outg = nc.dram_tensor("outg", [1, C], F32, kind="ExternalInput").ap()
    outD = nc.dram_tensor("out", [4, P, R, C], F32, kind="ExternalOutput").ap()

    with ExitStack() as ctx:
        tc = ctx.enter_context(tile.TileContext(nc))

        const = ctx.enter_context(tc.tile_pool(name="const", bufs=1))
        dram = ctx.enter_context(tc.tile_pool(name="dram", bufs=1, space="DRAM"))
        big = ctx.enter_context(tc.tile_pool(name="big", bufs=1))

        ident_f = const.tile([P, P], F32)
        make_identity(nc, ident_f[:])
        ident_b = const.tile([P, P], BF16)
        make_identity(nc, ident_b[:])
        nullk_s = const.tile([P, 2], BF16)
        nc.sync.dma_start(nullk_s[:], nullkD[:, :])
        nullv_s = const.tile([1, P], BF16)
        nc.sync.dma_start(nullv_s[:], nullv2[:, :])
        outg_s = const.tile([P, C], F32)
        nc.sync.dma_start(outg_s[:], outg.to_broadcast((P, C)))
        eps_t = const.tile([P, 1], F32)
        nc.vector.memset(eps_t[:], EPS)

        kT_s = big.tile([P, 4, CCOLS], BF16)           # 98.3KB/partition
        qT_d = dram.tile([P, 4, R, NQ], BF16)
        vD = dram.tile([P, 6, 8, 4, 2, 2, D], BF16)    # [j, tp, jc, g, hh, rp, d]
        aoT_d = dram.tile([P, 4, 6, 2, NQ], BF16)      # [e', g, t, rp, i]

        # round-robin PSUM->SBUF copy helper across DVE / ACT
        # (GPSIMD/Pool cannot access PSUM on real HW)
        _cp = [0]

        def copy_out(dst, src):
            k = _cp[0] % 2
            _cp[0] += 1
            if k == 0:
                nc.vector.tensor_copy(dst, src)
            else:
                nc.scalar.activation(dst, src, AF.Copy)

        # ---------------- Stage 1: LN + transposes + projections ----------
        with tc.tile_pool(name="w1", bufs=1) as wpool, \
             tc.tile_pool(name="s1", bufs=6) as s1, \
             tc.tile_pool(name="s1b", bufs=2) as s1b, \
             tc.tile_pool(name="s1v", bufs=1) as s1v, \
             tc.tile_pool(name="st1", bufs=8) as st1, \
             tc.tile_pool(name="pT", bufs=2, space="PSUM") as pT, \
             tc.tile_pool(name="pP", bufs=3, space="PSUM") as pP:

            wq_s = wpool.tile([P, 4, E], BF16)
            wk_s = wpool.tile([P, 4, E], BF16)
            wv_s = wpool.tile([P, 4, E], BF16)
            for cc in range(4):
                nc.sync.dma_start(wq_s[:, cc, :], wqT[cc * P : (cc + 1) * P, :])
                nc.sync.dma_start(wk_s[:, cc, :], wkT[cc * P : (cc + 1) * P, :])
                nc.sync.dma_start(wv_s[:, cc, :], wvT[cc * P : (cc + 1) * P, :])

            _vstP = [None]
            for rb in range(R):
                # ---- x side: LN + transpose + q projection ----
                xnT = s1b.tile([P, 4, NQ], BF16, tag="xnT")
                for ib in range(4):
                    xt = s1.tile([P, C], F32, tag="xt")
                    nc.sync.dma_start(xt[:], xR[ib * P : (ib + 1) * P, rb, :])
                    bst = st1.tile([P, 6], F32, tag="bst")
                    nc.vector.bn_stats(bst[:], xt[:])
                    mv = st1.tile([P, 2], F32, tag="mv")
                    nc.vector.bn_aggr(mv[:], bst[:])
                    stdv = st1.tile([P, 1], F32, tag="stdv")
                    nc.scalar.activation(stdv[:], mv[:, 1:2], AF.Sqrt, bias=eps_t[:])
                    inv = st1.tile([P, 1], F32, tag="inv")
                    nc.vector.reciprocal(inv[:], stdv[:])
                    negmi = st1.tile([P, 1], F32, tag="negmi")
                    nc.vector.tensor_mul(negmi[:], mv[:, 0:1], inv[:])
                    nc.scalar.mul(negmi[:], negmi[:], -1.0)
                    xn = s1.tile([P, C], BF16, tag="xn")
                    nc.scalar.activation(xn[:], xt[:], AF.Identity,
                                         bias=negmi[:], scale=inv[:])
                    psx = pT.tile([P, 4, P], BF16, tag="xtr")
                    for cc in range(4):
                        nc.tensor.transpose(psx[:, cc, :],
                                            xn[:, cc * P : (cc + 1) * P], ident_b[:])
                    copy_out(xnT[:, :, ib * P : (ib + 1) * P], psx[:])
                qst = s1b.tile([P, 4, NQ], BF16, tag="qst")
                for ec in range(4):
                    pq = pP.tile([P, NQ], F32, tag="proj")
                    for cc in range(4):
                        nc.tensor.matmul(pq[:],
                                         wq_s[:, cc, ec * P : (ec + 1) * P],
                                         xnT[:, cc, :],
                                         start=(cc == 0), stop=(cc == 3))
                    copy_out(qst[:, ec, :], pq[:])
                nc.sync.dma_start(qT_d[:, :, rb, :], qst[:])

                # ---- ctx side (r = rb): transpose + k/v projections ----
                cT = s1b.tile([P, 4, NKJ], BF16, tag="cT")
                for jc in range(8):
                    ct = s1.tile([P, C], F32, tag="ct")
                    nc.sync.dma_start(ct[:], ctxR[jc * P : (jc + 1) * P, rb, :])
                    cb = s1.tile([P, C], BF16, tag="cb")
                    nc.gpsimd.tensor_copy(cb[:], ct[:])
                    psc = pT.tile([P, 4, P], BF16, tag="ctr")
                    for cc in range(4):
                        nc.tensor.transpose(
                            psc[:, cc, :],
                            cb[:, cc * P : (cc + 1) * P], ident_b[:])
                    copy_out(cT[:, :, jc * P : (jc + 1) * P], psc[:])
                for jb in range(2):
                    for ec in range(4):
                        pk = pP.tile([P, NQ], F32, tag="proj")
                        for cc in range(4):
                            nc.tensor.matmul(
                                pk[:],
                                wk_s[:, cc, ec * P : (ec + 1) * P],
                                cT[:, cc, jb * NQ : (jb + 1) * NQ],
                                start=(cc == 0), stop=(cc == 3))
                        copy_out(
                            kT_s[:, ec, rb * NKJ + jb * NQ : rb * NKJ + (jb + 1) * NQ],
                            pk[:])
                if rb % 2 == 0:
                    vstP = s1v.tile([P, 8, 4, 2, 2, D], BF16, tag="vstP")
                    _vstP[0] = vstP
                else:
                    vstP = _vstP[0]
                rp = rb % 2
                for jc in range(8):
                    pv = pP.tile([P, E], F32, tag="proj")
                    for cc in range(4):
                        nc.tensor.matmul(
                            pv[:],
                            cT[:, cc, jc * P : (jc + 1) * P],
                            wv_s[:, cc, :],
                            start=(cc == 0), stop=(cc == 3))
                    pv4 = pv[:].rearrange("p (g hh d) -> p g hh d", hh=2, d=D)
                    for hh in range(2):
                        copy_out(vstP[:, jc, :, hh, rp, :], pv4[:, :, hh, :])
                if rb % 2 == 1:
                    nc.sync.dma_start(vD[:, rb // 2, :, :, :, :, :], vstP[:])

        # ---------------- Stage 2: attention ---------------------------
        with tc.tile_pool(name="s2q", bufs=2) as s2q, \
             tc.tile_pool(name="s2v", bufs=1) as s2v, \
             tc.tile_pool(name="s2e", bufs=4) as s2e, \
             tc.tile_pool(name="sPT", bufs=1) as sPT, \
             tc.tile_pool(name="s2a", bufs=1) as s2a, \
             tc.tile_pool(name="st2", bufs=2) as st2, \
             tc.tile_pool(name="srow", bufs=2) as srow, \
             tc.tile_pool(name="pS", bufs=3, space="PSUM") as pS, \
             tc.tile_pool(name="pPT", bufs=2, space="PSUM") as pPT, \
             tc.tile_pool(name="pAV", bufs=2, space="PSUM") as pAV, \
             tc.tile_pool(name="pDI", bufs=1, space="PSUM") as pDI:

            for g in range(4):
                qpair = s2q.tile([P, R, NQ], BF16, tag="qpair")
                nc.sync.dma_start(qpair[:], qT_d[:, g, :, :])
                vps = {}
                for half in range(2):
                    for hh in range(2):
                        vp = s2v.tile([P, 24, P], BF16,
                                      tag=f"vp{half}{hh}", name=f"vp{half}{hh}")
                        nc.sync.dma_start(
                            vp[:].rearrange("p (t jc) (rp d) -> p t jc rp d",
                                            jc=8, d=D),
                            vD[:, half * 3 : (half + 1) * 3, :, g, hh, :, :])
                        vps[(half, hh)] = vp
                PTs = [sPT.tile([P, 8, NQ], BF16, tag=f"PT{hh}", name=f"PT{hh}")
                       for hh in range(2)]
                PnTs = [srow.tile([1, NQ], BF16, tag=f"PnT{hh}", name=f"PnT{hh}")
                        for hh in range(2)]

                for ib in range(4):
                    # null sim for both heads: block-diag null-k, N=2
                    pn = pDI.tile([P, 2], F32, tag="tiny", name="pnull")
                    for r in range(R):
                        nc.tensor.matmul(
                            pn[:],
                            qpair[:, r, ib * P : (ib + 1) * P],
                            nullk_s[:, :],
                            start=(r == 0), stop=(r == R - 1))
                    en = st2.tile([P, 2], F32, tag="en")
                    nc.scalar.activation(en[:], pn[:], AF.Exp, scale=ALPHA)
                    sums = [[None, None], [None, None]]
                    es = [[None, None], [None, None]]
                    for jb in range(2):
                        pjs = []
                        for hh in range(2):
                            pj = pS.tile([P, NQ], F32, tag="sim", name=f"sim{hh}")
                            pjs.append(pj)
                        for r in range(R):
                            for hh in range(2):
                                pb = hh * D
                                nc.tensor.matmul(
                                    pjs[hh][:],
                                    qpair[pb : pb + D, r, ib * P : (ib + 1) * P],
                                    kT_s[pb : pb + D, g,
                                         r * NKJ + jb * NQ : r * NKJ + (jb + 1) * NQ],
                                    start=(r == 0), stop=(r == R - 1))
                        for hh in range(2):
                            e = s2e.tile([P, NQ], BF16, tag="e")
                            sacc = st2.tile([P, 1], F32, tag=f"sacc{hh}{jb}")
                            nc.scalar.activation(e[:], pjs[hh][:], AF.Exp,
                                                 scale=ALPHA, accum_out=sacc[:])
                            sums[hh][jb] = sacc
                            es[hh][jb] = e
                    den = st2.tile([P, 2], F32, tag="den")
                    for hh in range(2):
                        nc.vector.tensor_add(den[:, hh : hh + 1],
                                             sums[hh][0][:], sums[hh][1][:])
                    nc.vector.tensor_add(den[:], den[:], en[:])
                    dinv = st2.tile([P, 2], F32, tag="dinv2")
                    nc.vector.reciprocal(dinv[:], den[:])
                    # normalize null-P, transpose its row per head
                    ennorm = st2.tile([P, 2], F32, tag="ennorm")
                    nc.vector.tensor_mul(ennorm[:], en[:], dinv[:])
                    for hh in range(2):
                        pnt2 = pDI.tile([1, P], F32, tag="tiny", name="pnt2")
                        nc.tensor.transpose(pnt2[:], ennorm[:, hh : hh + 1],
                                            ident_f[:])
                        copy_out(PnTs[hh][:, ib * P : (ib + 1) * P], pnt2[:])
                    # normalize P along free dim, then transpose
                    for hh in range(2):
                        for jb in range(2):
                            e = es[hh][jb]
                            nc.vector.tensor_mul(
                                e[:], e[:],
                                dinv[:, hh : hh + 1].to_broadcast((P, NQ)))
                            psp = pPT.tile([P, 4, P], BF16, tag="ptr")
                            for c4 in range(4):
                                nc.tensor.transpose(
                                    psp[:, c4, :],
                                    e[:, c4 * P : (c4 + 1) * P], ident_b[:])
                            copy_out(
                                PTs[hh][:, jb * 4 : (jb + 1) * 4,
                                        ib * P : (ib + 1) * P],
                                psp[:])

                # attn @ v, r-parity packed (M=128), 1/den folded into copy
                for hh in range(2):
                    pb = hh * D
                    avst = s2a.tile([P, 6, NQ], BF16, tag=f"avst{hh}")
                    for t in range(6):
                        vp = vps[(t // 3, hh)]
                        tt = t % 3
                        pav = pAV.tile([P, NQ], F32, tag="pav")
                        for jc in range(8):
                            nc.tensor.matmul(
                                pav[:],
                                vp[:, tt * 8 + jc, :],
                                PTs[hh][:, jc, :],
                                start=(jc == 0), stop=False)
                        nc.tensor.matmul(
                            pav[:], nullv_s[:, :], PnTs[hh][:],
                            start=False, stop=True)
                        copy_out(avst[:, t, :], pav[:])
                    nc.sync.dma_start(aoT_d[pb : pb + D, g, :, 0, :],
                                      avst[0:D, :, :])
                    nc.sync.dma_start(aoT_d[pb : pb + D, g, :, 1, :],
                                      avst[D : 2 * D, :, :])

        # ---------------- Stage 3: out projection + final LN ------------
        with tc.tile_pool(name="w3", bufs=1) as w3, \
             tc.tile_pool(name="s3", bufs=3) as s3, \
             tc.tile_pool(name="st3", bufs=8) as st3, \
             tc.tile_pool(name="p3", bufs=3, space="PSUM") as p3:

            wo_s = w3.tile([P, 4, C], BF16)
            for ec in range(4):
                nc.sync.dma_start(wo_s[:, ec, :], woT[ec * P : (ec + 1) * P, :])

            for r in range(R):
                at = s3.tile([P, 4, NQ], BF16, tag="at")
                nc.sync.dma_start(at[:], aoT_d[:, :, r // 2, r % 2, :])
                otb = s3.tile([P, 4, C], F32, tag="otb")
                for ib in range(4):
                    pf = p3.tile([P, C], F32, tag="pf")
                    for ec in range(4):
                        nc.tensor.matmul(
                            pf[:],
                            at[:, ec, ib * P : (ib + 1) * P],
                            wo_s[:, ec, :],
                            start=(ec == 0), stop=(ec == 3))
                    bst = st3.tile([P, 6], F32, tag="bst3")
                    nc.vector.bn_stats(bst[:], pf[:])
                    mv = st3.tile([P, 2], F32, tag="mv3")
                    nc.vector.bn_aggr(mv[:], bst[:])
                    stdv = st3.tile([P, 1], F32, tag="stdv3")
                    nc.scalar.activation(stdv[:], mv[:, 1:2], AF.Sqrt, bias=eps_t[:])
                    inv = st3.tile([P, 1], F32, tag="inv3")
                    nc.vector.reciprocal(inv[:], stdv[:])
                    negmi = st3.tile([P, 1], F32, tag="negmi3")
                    nc.vector.tensor_mul(negmi[:], mv[:, 0:1], inv[:])
                    nc.scalar.mul(negmi[:], negmi[:], -1.0)
                    t1 = s3.tile([P, C], F32, tag="t1")
                    nc.scalar.activation(t1[:], pf[:], AF.Identity,
                                         bias=negmi[:], scale=inv[:])
                    nc.gpsimd.tensor_mul(otb[:, ib, :], t1[:], outg_s[:, :])
                nc.sync.dma_start(
                    outD[:, :, r, :].rearrange("a b c -> b a c"), otb[:])

    nc.compile()
    return nc


def _make_in_maps(x, context, norm_g, to_q_w, to_kv_w, null_kv, to_out_w, out_norm_g):
    import ml_dtypes
    BF = ml_dtypes.bfloat16

    scale = (D ** -0.5) / ALPHA * (R ** -0.5)
    wq = np.ascontiguousarray((to_q_w * norm_g[None, :] * scale).T).astype(BF)
    wk = np.ascontiguousarray(to_kv_w[:E].T).astype(BF)
    wv = np.ascontiguousarray(to_kv_w[E:].T).astype(BF)
    wo = np.ascontiguousarray(to_out_w.T).astype(BF)
    nullkD_a = np.zeros((P, 2), dtype=BF)
    nullkD_a[0:D, 0] = null_kv[0].astype(BF)
    nullkD_a[D : 2 * D, 1] = null_kv[0].astype(BF)
    nullv2_a = np.concatenate([null_kv[1], null_kv[1]]).reshape(1, P).astype(BF)
    outg_a = np.ascontiguousarray(out_norm_g.reshape(1, C)).astype(np.float32)

    in_maps = []
    for core in range(8):
        bi, half = core // 2, core % 2
        xs = x[bi, half * NQ : (half + 1) * NQ]          # [512, 12, 512] view
        cs = context[bi]                                  # [1024, 12, 512] view
        in_maps.append(dict(
            xR=xs, ctxR=cs, wqT=wq, wkT=wk, wvT=wv, woT=wo,
            nullkD=nullkD_a, nullv2=nullv2_a, outg=outg_a))
    return in_maps


def _unshard_core0(out):
    return out.reshape(NQ, R, C)


def _expected_core0(expected):
    return expected[0, 0:NQ]


def kernel(x, context, norm_g, to_q_w, to_kv_w, null_kv, to_out_w, out_norm_g):
    from concourse.bass_utils import run_bass_kernel_spmd

    x = np.asarray(x, dtype=np.float32)
    context = np.asarray(context, dtype=np.float32)
    norm_g = np.asarray(norm_g, dtype=np.float32)
    to_q_w = np.asarray(to_q_w, dtype=np.float32)
    to_kv_w = np.asarray(to_kv_w, dtype=np.float32)
    null_kv = np.asarray(null_kv, dtype=np.float32)
    to_out_w = np.asarray(to_out_w, dtype=np.float32)
    out_norm_g = np.asarray(out_norm_g, dtype=np.float32)

    if "nc" not in _CACHE:
        _CACHE["nc"] = _build_program()
    nc = _CACHE["nc"]

    in_maps = _make_in_maps(
        x, context, norm_g, to_q_w, to_kv_w, null_kv, to_out_w, out_norm_g)

    trace = bool(int(os.environ.get("KERNEL_TRACE", "0")))
    res = run_bass_kernel_spmd(nc, in_maps, list(range(8)), trace=trace)
    _CACHE["last_exec_ns"] = res.exec_time_ns

    full = np.empty((B, N, R, C), dtype=np.float32)
    for core in range(8):
        bi, half = core // 2, core % 2
        o = res.results[core]["out"]                      # [512, 12, 512]
        full[bi, half * NQ : (half + 1) * NQ] = o.reshape(NQ, R, C)
    return full
